# revision 8
# baseline (speedup 1.0000x reference)
"""Trainium2 Bass kernel for nn_MemoryImageUpdater (topk_masking).

Contract: kernel(**inputs) takes FULL unsharded inputs (numpy), returns the
FULL output tuple matching reference.reference():
  (memory_image, memory_state, score_state, importance, write_mask, output_mask)

Sharding: data-parallel over batch B=16 across 8 NeuronCores (2 rows/core).
Each core processes its 2 batch rows in a [128, 4096] layout: row 0 on
partitions 0..63, row 1 on partitions 64..127, 4096 f32 per partition
(512*512 = 64*4096 pixels per image plane).

Algorithm per core (all bit-exact vs the f32 jax reference):
  Pass A: importance = sum_c cp[c]*w[c] (sequential order, fused mul-add),
          gate = (max_c cp[c] > bg), decayed = ps*0.9, thr = decayed+0.05,
          write_mask = imp*gate > thr, score = where(wm, imp*gate, decayed).
  Select: exact per-row top-k threshold T (k = 65536 of 262144) via MSB-first
          radix descent on the positive-f32 bit pattern: 30 rounds of
          t |= (count(score >= bitcast(t | 1<<b)) >= k) << b, where the count
          is one fused DVE tensor_tensor_reduce + two tiny PE matmuls
          (per-row partition sum + broadcast back).  Positive-f32 ordering
          equals bit-pattern ordering, so comparisons run in f32.
  Pass B: memory_state = where(wm, current, prev) per channel.
  Pass C: output_mask = score >= T; memory_image = memory_state * mask.
"""

import os
import numpy as np
from contextlib import ExitStack

import concourse.tile as tile
from concourse import bacc, mybir
from concourse import bass_utils

B, C, NCL = 16, 3, 8
H, W = 512, 512
NCORES = 8
RPC = B // NCORES           # batch rows per core
P = 128                     # SBUF partitions
PR = P // RPC               # partitions per row = 64
F = (H * W) // PR           # free dim = 4096
KEEP = 65536                # ceil(0.25 * H * W)
TOP_BIT = 29                # scores in [0, 1): bit patterns < 0x3F800000

f32 = mybir.dt.float32
i32 = mybir.dt.int32
u8 = mybir.dt.uint8
Alu = mybir.AluOpType

_cached = {}


def _pair_ap(dram, c=None):
    """DRAM AP for both rows of one channel plane, [RPC, PR, F] element order.

    DMA pairs this with a [128, F] SBUF tile AP: row r lands on partitions
    r*64..(r+1)*64.  Dims are left unmerged (strides differ when a channel is
    sliced out); dma_start's AP balancing matches the element orders.
    """
    return dram.ap() if c is None else dram.ap()[:, c]


def _emit(nc, tc, t):
    with ExitStack() as ctx:
        # Pools open for the whole kernel (small persistents only).
        cpool = ctx.enter_context(tc.tile_pool(name="consts", bufs=1))
        perm = ctx.enter_context(tc.tile_pool(name="perm", bufs=1))
        psp = ctx.enter_context(tc.tile_pool(name="psum", bufs=2, space="PSUM"))

        wb = cpool.tile([P, NCL], f32)
        nc.sync.dma_start(wb[:], t["wb"].ap())
        sel = cpool.tile([P, RPC], f32)
        nc.sync.dma_start(sel[:], t["sel"].ap())
        sel2 = cpool.tile([RPC, P], f32)
        nc.sync.dma_start(sel2[:], t["sel2"].ap())
        c005 = cpool.tile([P, 1], f32)
        nc.vector.memset(c005[:], 0.05)

        score = perm.tile([P, F], f32)      # prev_scores -> decayed -> score_state
        wm = perm.tile([P, F], u8)
        tbits = perm.tile([P, 1], i32)

        # ---- Pass A (scoped pools: space returns before Pass B) ----
        with tc.tile_pool(name="cp", bufs=3) as cp_pool, \
             tc.tile_pool(name="pa", bufs=1) as apool:
            bg_t = apool.tile([P, F], f32)
            nc.sync.dma_start(bg_t[:], _pair_ap(t["bg"]))
            nc.sync.dma_start(score[:], _pair_ap(t["ps"]))

            imp = apool.tile([P, F], f32)
            m8 = apool.tile([P, F], f32)
            for c in range(NCL):
                cpt = cp_pool.tile([P, F], f32, tag="cp")
                nc.sync.dma_start(cpt[:], _pair_ap(t["cp"], c))
                if c == 0:
                    nc.vector.tensor_tensor(
                        imp[:], cpt[:], wb[:, 0:1].broadcast_to([P, F]), Alu.mult
                    )
                    nc.scalar.copy(m8[:], cpt[:])
                else:
                    nc.vector.scalar_tensor_tensor(
                        out=imp[:], in0=cpt[:], scalar=wb[:, c : c + 1], in1=imp[:],
                        op0=Alu.mult, op1=Alu.add,
                    )
                    nc.vector.tensor_tensor(m8[:], m8[:], cpt[:], Alu.max)

            gate = apool.tile([P, F], f32)
            nc.vector.tensor_tensor(gate[:], m8[:], bg_t[:], Alu.is_gt)
            nc.vector.tensor_tensor(imp[:], imp[:], gate[:], Alu.mult)
            nc.sync.dma_start(_pair_ap(t["o_imp"]), imp[:])

            nc.scalar.mul(score[:], score[:], 0.9)      # score := decayed
            thr = apool.tile([P, F], f32)
            nc.scalar.activation(
                thr[:], score[:], mybir.ActivationFunctionType.Identity,
                bias=c005[:, 0:1], scale=1.0,
            )
            nc.vector.tensor_tensor(wm[:], imp[:], thr[:], Alu.is_gt)
            nc.sync.dma_start(_pair_ap(t["o_wm"]), wm[:])
            # score := score_state = where(wm, imp, decayed)
            nc.vector.copy_predicated(score[:], wm[:], imp[:])
            nc.sync.dma_start(_pair_ap(t["o_score"]), score[:])

        # ---- Pass B + top-k select + Pass C ----
        with tc.tile_pool(name="ms", bufs=1) as msp, \
             tc.tile_pool(name="cur", bufs=2) as curp, \
             tc.tile_pool(name="bs", bufs=1) as bsp, \
             tc.tile_pool(name="bst", bufs=2) as bstp:
            # Pass B: memory_state (independent of the top-k select)
            ms_tiles = []
            for c in range(C):
                ms_c = msp.tile([P, F], f32, tag=f"ms{c}")
                nc.sync.dma_start(ms_c[:], _pair_ap(t["pm"], c))
                cur_t = curp.tile([P, F], f32, tag="cur")
                nc.sync.dma_start(cur_t[:], _pair_ap(t["cur"], c))
                nc.vector.copy_predicated(ms_c[:], wm[:], cur_t[:])
                nc.sync.dma_start(_pair_ap(t["o_ms"], c), ms_c[:])
                ms_tiles.append(ms_c)

            # Exact per-row top-k threshold: MSB-first radix descent
            nbits = int(os.environ.get("KERNEL_NBITS", "30"))
            if nbits < 30:
                nc.vector.memset(tbits[:], 0x3F340000)  # debug: skip descent tail
            else:
                nc.vector.memset(tbits[:], 0)
            for b in range(TOP_BIT, TOP_BIT - nbits, -1):
                cand = bstp.tile([P, 1], i32, tag="cand")
                nc.vector.tensor_scalar(cand[:], tbits[:], 1 << b, None, Alu.bitwise_or)
                scratch = bsp.tile([P, F], f32, tag="scr")
                cpart = bstp.tile([P, 1], f32, tag="cpart")
                # fused per-partition count: out = (score >= T_cand), accum = sum
                # (tensor_tensor_reduce crashes the exec unit on this runtime;
                # scalar_tensor_tensor with a [P,1] scalar AP + accum_out works)
                nc.vector.scalar_tensor_tensor(
                    out=scratch[:], in0=score[:], scalar=cand[:].bitcast(f32),
                    in1=score[:], op0=Alu.is_ge, op1=Alu.bypass,
                    accum_out=cpart[:],
                )
                pc2 = psp.tile([RPC, 1], f32, tag="pc2")
                nc.tensor.matmul(pc2[:], sel[:], cpart[:], start=True, stop=True)
                c2 = bstp.tile([RPC, 1], f32, tag="c2")
                nc.vector.tensor_copy(c2[:], pc2[:])
                pball = psp.tile([P, 1], f32, tag="pball")
                nc.tensor.matmul(pball[:], sel2[:], c2[:], start=True, stop=True)
                cond = bstp.tile([P, 1], i32, tag="cond")
                nc.vector.tensor_scalar(cond[:], pball[:], float(KEEP), None, Alu.is_ge)
                bitv = bstp.tile([P, 1], i32, tag="bitv")
                nc.vector.tensor_scalar(bitv[:], cond[:], b, None, Alu.logical_shift_left)
                nc.vector.tensor_tensor(tbits[:], tbits[:], bitv[:], Alu.bitwise_or)

            # Pass C: output mask + memory_image
            maskf = bsp.tile([P, F], f32, tag="maskf")
            nc.vector.tensor_tensor(
                maskf[:], score[:], tbits[:].bitcast(f32).broadcast_to([P, F]), Alu.is_ge
            )
            om8 = bsp.tile([P, F], u8, tag="om8")
            nc.scalar.copy(om8[:], maskf[:])
            nc.sync.dma_start(_pair_ap(t["o_om"]), om8[:])
            for c in range(C):
                ms_c = ms_tiles[c]
                nc.vector.tensor_tensor(ms_c[:], ms_c[:], maskf[:], Alu.mult)
                nc.sync.dma_start(_pair_ap(t["o_mi"], c), ms_c[:])


def build():
    if "nc" in _cached:
        return _cached["nc"]
    nc = bacc.Bacc("TRN2", target_bir_lowering=False, debug=False, enable_asserts=True)
    t = {
        "cur": nc.dram_tensor("cur", [RPC, C, PR, F], f32, kind="ExternalInput"),
        "cp": nc.dram_tensor("cp", [RPC, NCL, PR, F], f32, kind="ExternalInput"),
        "bg": nc.dram_tensor("bg", [RPC, PR, F], f32, kind="ExternalInput"),
        "pm": nc.dram_tensor("pm", [RPC, C, PR, F], f32, kind="ExternalInput"),
        "ps": nc.dram_tensor("ps", [RPC, PR, F], f32, kind="ExternalInput"),
        "wb": nc.dram_tensor("wb", [P, NCL], f32, kind="ExternalInput"),
        "sel": nc.dram_tensor("sel", [P, RPC], f32, kind="ExternalInput"),
        "sel2": nc.dram_tensor("sel2", [RPC, P], f32, kind="ExternalInput"),
        "o_mi": nc.dram_tensor("o_mi", [RPC, C, PR, F], f32, kind="ExternalOutput"),
        "o_ms": nc.dram_tensor("o_ms", [RPC, C, PR, F], f32, kind="ExternalOutput"),
        "o_score": nc.dram_tensor("o_score", [RPC, PR, F], f32, kind="ExternalOutput"),
        "o_imp": nc.dram_tensor("o_imp", [RPC, PR, F], f32, kind="ExternalOutput"),
        "o_wm": nc.dram_tensor("o_wm", [RPC, PR, F], u8, kind="ExternalOutput"),
        "o_om": nc.dram_tensor("o_om", [RPC, PR, F], u8, kind="ExternalOutput"),
    }
    with tile.TileContext(nc) as tc:
        _emit(nc, tc, t)
    nc.compile()
    _cached["nc"] = nc
    return nc


def make_in_maps(current_image, class_probs, background_prob, prev_memory,
                 prev_scores, class_weights):
    wb = np.ascontiguousarray(
        np.broadcast_to(class_weights.reshape(1, NCL).astype(np.float32), (P, NCL))
    )
    sel = np.zeros((P, RPC), np.float32)
    sel2 = np.zeros((RPC, P), np.float32)
    for r in range(RPC):
        sel[r * PR : (r + 1) * PR, r] = 1.0
        sel2[r, r * PR : (r + 1) * PR] = 1.0
    in_maps = []
    for i in range(NCORES):
        b0 = i * RPC
        sl = slice(b0, b0 + RPC)
        in_maps.append({
            "cur": np.ascontiguousarray(current_image[sl]).reshape(RPC, C, PR, F),
            "cp": np.ascontiguousarray(class_probs[sl]).reshape(RPC, NCL, PR, F),
            "bg": np.ascontiguousarray(background_prob[sl]).reshape(RPC, PR, F),
            "pm": np.ascontiguousarray(prev_memory[sl]).reshape(RPC, C, PR, F),
            "ps": np.ascontiguousarray(prev_scores[sl]).reshape(RPC, PR, F),
            "wb": wb, "sel": sel, "sel2": sel2,
        })
    return in_maps


def assemble(results):
    def gather(name, chans, dtype=None):
        parts = [results[i][name].reshape((RPC, chans, H, W)) for i in range(NCORES)]
        out = np.concatenate(parts, axis=0)
        return out if dtype is None else out.astype(dtype)

    return (
        gather("o_mi", C),
        gather("o_ms", C),
        gather("o_score", 1),
        gather("o_imp", 1),
        gather("o_wm", 1, np.bool_),
        gather("o_om", 1, np.bool_),
    )


def kernel(current_image, class_probs, background_prob, prev_memory,
           prev_scores, class_weights, _trace=False):
    nc = build()
    in_maps = make_in_maps(current_image, class_probs, background_prob,
                           prev_memory, prev_scores, class_weights)
    res = bass_utils.run_bass_kernel_spmd(
        nc, in_maps, core_ids=list(range(NCORES)),
        trace=_trace or bool(os.environ.get("KERNEL_TRACE")),
    )
    _cached["last_result"] = res
    return assemble(res.results)


# revision 9
# speedup vs baseline: 2.5473x; 2.5473x over previous
"""Trainium2 Bass kernel for nn_MemoryImageUpdater (topk_masking).

Contract: kernel(**inputs) takes FULL unsharded inputs (numpy), returns the
FULL output tuple matching reference.reference():
  (memory_image, memory_state, score_state, importance, write_mask, output_mask)

Sharding: data-parallel over batch B=16 across 8 NeuronCores (2 rows/core).
Each core processes its 2 batch rows in a [128, 4096] layout: row 0 on
partitions 0..63, row 1 on partitions 64..127, 4096 f32 per partition
(512*512 = 64*4096 pixels per image plane).

Algorithm per core (all bit-exact vs the f32 jax reference):
  Pass A: importance = sum_c cp[c]*w[c] (sequential order, fused mul-add),
          gate = (max_c cp[c] > bg), decayed = ps*0.9, thr = decayed+0.05,
          write_mask = imp*gate > thr, score = where(wm, imp*gate, decayed).
  Select: exact per-row top-k threshold T (k = 65536 of 262144) via MSB-first
          radix descent on the positive-f32 bit pattern: 30 rounds of
          t |= (count(score >= bitcast(t | 1<<b)) >= k) << b, where the count
          is one fused DVE tensor_tensor_reduce + two tiny PE matmuls
          (per-row partition sum + broadcast back).  Positive-f32 ordering
          equals bit-pattern ordering, so comparisons run in f32.
  Pass B: memory_state = where(wm, current, prev) per channel.
  Pass C: output_mask = score >= T; memory_image = memory_state * mask.
"""

import os
import numpy as np
from contextlib import ExitStack

import concourse.tile as tile
from concourse import bacc, mybir
from concourse import bass_utils

B, C, NCL = 16, 3, 8
H, W = 512, 512
NCORES = 8
RPC = B // NCORES           # batch rows per core
P = 128                     # SBUF partitions
PR = P // RPC               # partitions per row = 64
F = (H * W) // PR           # free dim = 4096
KEEP = 65536                # ceil(0.25 * H * W)
TOP_BIT = 29                # scores in [0, 1): bit patterns < 0x3F800000

f32 = mybir.dt.float32
i32 = mybir.dt.int32
u8 = mybir.dt.uint8
Alu = mybir.AluOpType

_cached = {}


def _pair_ap(dram, c=None):
    """DRAM AP of one channel plane as contiguous [128, F] (row r of the core's
    batch pair lives on partitions r*64..(r+1)*64).  Channel-major DRAM layout
    keeps the outer AP dim at 128 so the DMA splits across all 16 SDMA slots."""
    return dram.ap() if c is None else dram.ap()[c]


def _emit(nc, tc, t):
    with ExitStack() as ctx:
        # Pools open for the whole kernel (small persistents only).
        cpool = ctx.enter_context(tc.tile_pool(name="consts", bufs=1))
        perm = ctx.enter_context(tc.tile_pool(name="perm", bufs=1))
        psp = ctx.enter_context(tc.tile_pool(name="psum", bufs=2, space="PSUM"))

        wb = cpool.tile([P, NCL], f32)
        nc.sync.dma_start(wb[:], t["wb"].ap())
        sel = cpool.tile([P, RPC], f32)
        nc.sync.dma_start(sel[:], t["sel"].ap())
        sel2 = cpool.tile([RPC, P], f32)
        nc.sync.dma_start(sel2[:], t["sel2"].ap())
        c005 = cpool.tile([P, 1], f32)
        nc.vector.memset(c005[:], 0.05)

        score = perm.tile([P, F], f32)      # prev_scores -> decayed -> score_state
        wm = perm.tile([P, F], u8)
        tbits = perm.tile([P, 1], i32)

        # ---- Pass A (scoped pools: space returns before Pass B) ----
        with tc.tile_pool(name="cp", bufs=4) as cp_pool, \
             tc.tile_pool(name="pa", bufs=1) as apool:
            bg_t = apool.tile([P, F], f32)
            nc.sync.dma_start(bg_t[:], _pair_ap(t["bg"]))
            nc.scalar.dma_start(score[:], _pair_ap(t["ps"]))

            imp = apool.tile([P, F], f32)
            m8 = apool.tile([P, F], f32)
            for c in range(NCL):
                cpt = cp_pool.tile([P, F], f32, tag="cp")
                dma_eng = nc.sync if c % 2 == 0 else nc.scalar
                dma_eng.dma_start(cpt[:], _pair_ap(t["cp"], c))
                if c == 0:
                    nc.vector.tensor_tensor(
                        imp[:], cpt[:], wb[:, 0:1].broadcast_to([P, F]), Alu.mult
                    )
                    nc.scalar.copy(m8[:], cpt[:])
                else:
                    nc.vector.scalar_tensor_tensor(
                        out=imp[:], in0=cpt[:], scalar=wb[:, c : c + 1], in1=imp[:],
                        op0=Alu.mult, op1=Alu.add,
                    )
                    nc.vector.tensor_tensor(m8[:], m8[:], cpt[:], Alu.max)

            gate = apool.tile([P, F], f32)
            nc.vector.tensor_tensor(gate[:], m8[:], bg_t[:], Alu.is_gt)
            nc.vector.tensor_tensor(imp[:], imp[:], gate[:], Alu.mult)
            nc.sync.dma_start(_pair_ap(t["o_imp"]), imp[:])

            nc.scalar.mul(score[:], score[:], 0.9)      # score := decayed
            thr = apool.tile([P, F], f32)
            nc.scalar.activation(
                thr[:], score[:], mybir.ActivationFunctionType.Identity,
                bias=c005[:, 0:1], scale=1.0,
            )
            nc.vector.tensor_tensor(wm[:], imp[:], thr[:], Alu.is_gt)
            nc.scalar.dma_start(_pair_ap(t["o_wm"]), wm[:])
            # score := score_state = where(wm, imp, decayed)
            nc.vector.copy_predicated(score[:], wm[:], imp[:])
            nc.sync.dma_start(_pair_ap(t["o_score"]), score[:])

        # ---- Pass B + top-k select + Pass C ----
        with tc.tile_pool(name="ms", bufs=1) as msp, \
             tc.tile_pool(name="cur", bufs=2) as curp, \
             tc.tile_pool(name="bs", bufs=1) as bsp, \
             tc.tile_pool(name="bst", bufs=2) as bstp:
            # Pass B: memory_state (independent of the top-k select)
            ms_tiles = []
            for c in range(C):
                ms_c = msp.tile([P, F], f32, tag=f"ms{c}")
                nc.sync.dma_start(ms_c[:], _pair_ap(t["pm"], c))
                cur_t = curp.tile([P, F], f32, tag="cur")
                nc.scalar.dma_start(cur_t[:], _pair_ap(t["cur"], c))
                nc.vector.copy_predicated(ms_c[:], wm[:], cur_t[:])
                nc.sync.dma_start(_pair_ap(t["o_ms"], c), ms_c[:])
                ms_tiles.append(ms_c)

            # Exact per-row top-k threshold: MSB-first radix descent
            nbits = int(os.environ.get("KERNEL_NBITS", "30"))
            if nbits < 30:
                nc.vector.memset(tbits[:], 0x3F340000)  # debug: skip descent tail
            else:
                nc.vector.memset(tbits[:], 0)
            for b in range(TOP_BIT, TOP_BIT - nbits, -1):
                cand = bstp.tile([P, 1], i32, tag="cand")
                nc.vector.tensor_scalar(cand[:], tbits[:], 1 << b, None, Alu.bitwise_or)
                scratch = bsp.tile([P, F], f32, tag="scr")
                cpart = bstp.tile([P, 1], f32, tag="cpart")
                # fused per-partition count: out = (score >= T_cand), accum = sum
                # (tensor_tensor_reduce crashes the exec unit on this runtime;
                # scalar_tensor_tensor with a [P,1] scalar AP + accum_out works)
                nc.vector.scalar_tensor_tensor(
                    out=scratch[:], in0=score[:], scalar=cand[:].bitcast(f32),
                    in1=score[:], op0=Alu.is_ge, op1=Alu.bypass,
                    accum_out=cpart[:],
                )
                pc2 = psp.tile([RPC, 1], f32, tag="pc2")
                nc.tensor.matmul(pc2[:], sel[:], cpart[:], start=True, stop=True)
                c2 = bstp.tile([RPC, 1], f32, tag="c2")
                nc.vector.tensor_copy(c2[:], pc2[:])
                pball = psp.tile([P, 1], f32, tag="pball")
                nc.tensor.matmul(pball[:], sel2[:], c2[:], start=True, stop=True)
                cond = bstp.tile([P, 1], i32, tag="cond")
                nc.vector.tensor_scalar(cond[:], pball[:], float(KEEP), None, Alu.is_ge)
                bitv = bstp.tile([P, 1], i32, tag="bitv")
                nc.vector.tensor_scalar(bitv[:], cond[:], b, None, Alu.logical_shift_left)
                nc.vector.tensor_tensor(tbits[:], tbits[:], bitv[:], Alu.bitwise_or)

            # Pass C: output mask + memory_image
            maskf = bsp.tile([P, F], f32, tag="maskf")
            nc.vector.tensor_tensor(
                maskf[:], score[:], tbits[:].bitcast(f32).broadcast_to([P, F]), Alu.is_ge
            )
            om8 = bsp.tile([P, F], u8, tag="om8")
            nc.scalar.copy(om8[:], maskf[:])
            nc.scalar.dma_start(_pair_ap(t["o_om"]), om8[:])
            for c in range(C):
                ms_c = ms_tiles[c]
                nc.vector.tensor_tensor(ms_c[:], ms_c[:], maskf[:], Alu.mult)
                dma_eng = nc.sync if c % 2 == 0 else nc.scalar
                dma_eng.dma_start(_pair_ap(t["o_mi"], c), ms_c[:])


def build():
    if "nc" in _cached:
        return _cached["nc"]
    nc = bacc.Bacc("TRN2", target_bir_lowering=False, debug=False, enable_asserts=True)
    t = {
        "cur": nc.dram_tensor("cur", [C, P, F], f32, kind="ExternalInput"),
        "cp": nc.dram_tensor("cp", [NCL, P, F], f32, kind="ExternalInput"),
        "bg": nc.dram_tensor("bg", [P, F], f32, kind="ExternalInput"),
        "pm": nc.dram_tensor("pm", [C, P, F], f32, kind="ExternalInput"),
        "ps": nc.dram_tensor("ps", [P, F], f32, kind="ExternalInput"),
        "wb": nc.dram_tensor("wb", [P, NCL], f32, kind="ExternalInput"),
        "sel": nc.dram_tensor("sel", [P, RPC], f32, kind="ExternalInput"),
        "sel2": nc.dram_tensor("sel2", [RPC, P], f32, kind="ExternalInput"),
        "o_mi": nc.dram_tensor("o_mi", [C, P, F], f32, kind="ExternalOutput"),
        "o_ms": nc.dram_tensor("o_ms", [C, P, F], f32, kind="ExternalOutput"),
        "o_score": nc.dram_tensor("o_score", [P, F], f32, kind="ExternalOutput"),
        "o_imp": nc.dram_tensor("o_imp", [P, F], f32, kind="ExternalOutput"),
        "o_wm": nc.dram_tensor("o_wm", [P, F], u8, kind="ExternalOutput"),
        "o_om": nc.dram_tensor("o_om", [P, F], u8, kind="ExternalOutput"),
    }
    with tile.TileContext(nc) as tc:
        _emit(nc, tc, t)
    nc.compile()
    _cached["nc"] = nc
    return nc


def make_in_maps(current_image, class_probs, background_prob, prev_memory,
                 prev_scores, class_weights):
    wb = np.ascontiguousarray(
        np.broadcast_to(class_weights.reshape(1, NCL).astype(np.float32), (P, NCL))
    )
    sel = np.zeros((P, RPC), np.float32)
    sel2 = np.zeros((RPC, P), np.float32)
    for r in range(RPC):
        sel[r * PR : (r + 1) * PR, r] = 1.0
        sel2[r, r * PR : (r + 1) * PR] = 1.0
    in_maps = []
    for i in range(NCORES):
        b0 = i * RPC
        sl = slice(b0, b0 + RPC)
        def chmajor(x, nch):
            # [RPC, nch, 512, 512] -> [nch, RPC*PR, F] channel-major contiguous
            return np.ascontiguousarray(
                x.reshape(RPC, nch, P // RPC, F).transpose(1, 0, 2, 3)
            ).reshape(nch, P, F)

        in_maps.append({
            "cur": chmajor(current_image[sl], C),
            "cp": chmajor(class_probs[sl], NCL),
            "bg": np.ascontiguousarray(background_prob[sl]).reshape(P, F),
            "pm": chmajor(prev_memory[sl], C),
            "ps": np.ascontiguousarray(prev_scores[sl]).reshape(P, F),
            "wb": wb, "sel": sel, "sel2": sel2,
        })
    return in_maps


def assemble(results):
    def gather(name, chans, dtype=None):
        parts = []
        for i in range(NCORES):
            a = results[i][name]
            if chans > 1:
                # [chans, P, F] channel-major -> [RPC, chans, 512, 512]
                a = a.reshape(chans, RPC, PR, F).transpose(1, 0, 2, 3)
            a = a.reshape(RPC, chans, H, W)
            parts.append(a)
        out = np.concatenate(parts, axis=0)
        return out if dtype is None else out.astype(dtype)

    return (
        gather("o_mi", C),
        gather("o_ms", C),
        gather("o_score", 1),
        gather("o_imp", 1),
        gather("o_wm", 1, np.bool_),
        gather("o_om", 1, np.bool_),
    )


def kernel(current_image, class_probs, background_prob, prev_memory,
           prev_scores, class_weights, _trace=False):
    nc = build()
    in_maps = make_in_maps(current_image, class_probs, background_prob,
                           prev_memory, prev_scores, class_weights)
    res = bass_utils.run_bass_kernel_spmd(
        nc, in_maps, core_ids=list(range(NCORES)),
        trace=_trace or bool(os.environ.get("KERNEL_TRACE")),
    )
    _cached["last_result"] = res
    return assemble(res.results)


# revision 10
# speedup vs baseline: 2.7402x; 1.0757x over previous
"""Trainium2 Bass kernel for nn_MemoryImageUpdater (topk_masking).

Contract: kernel(**inputs) takes FULL unsharded inputs (numpy), returns the
FULL output tuple matching reference.reference():
  (memory_image, memory_state, score_state, importance, write_mask, output_mask)

Sharding: data-parallel over batch B=16 across 8 NeuronCores (2 rows/core).
Each core processes its 2 batch rows in a [128, 4096] layout: row 0 on
partitions 0..63, row 1 on partitions 64..127, 4096 f32 per partition
(512*512 = 64*4096 pixels per image plane).

Algorithm per core (all bit-exact vs the f32 jax reference):
  Pass A: importance = sum_c cp[c]*w[c] (sequential order, fused mul-add),
          gate = (max_c cp[c] > bg), decayed = ps*0.9, thr = decayed+0.05,
          write_mask = imp*gate > thr, score = where(wm, imp*gate, decayed).
  Select: exact per-row top-k threshold T (k = 65536 of 262144) via MSB-first
          radix descent on the positive-f32 bit pattern: 30 rounds of
          t |= (count(score >= bitcast(t | 1<<b)) >= k) << b, where the count
          is one fused DVE tensor_tensor_reduce + two tiny PE matmuls
          (per-row partition sum + broadcast back).  Positive-f32 ordering
          equals bit-pattern ordering, so comparisons run in f32.
  Pass B: memory_state = where(wm, current, prev) per channel.
  Pass C: output_mask = score >= T; memory_image = memory_state * mask.
"""

import os
import numpy as np
from contextlib import ExitStack

import concourse.tile as tile
from concourse import bacc, mybir
from concourse import bass_utils

B, C, NCL = 16, 3, 8
H, W = 512, 512
NCORES = 8
RPC = B // NCORES           # batch rows per core
P = 128                     # SBUF partitions
PR = P // RPC               # partitions per row = 64
F = (H * W) // PR           # free dim = 4096
KEEP = 65536                # ceil(0.25 * H * W)
TOP_BIT = 29                # scores in [0, 1): bit patterns < 0x3F800000

f32 = mybir.dt.float32
i32 = mybir.dt.int32
u8 = mybir.dt.uint8
Alu = mybir.AluOpType

_cached = {}


def _pair_ap(dram, c=None):
    """DRAM AP of one channel plane as contiguous [128, F] (row r of the core's
    batch pair lives on partitions r*64..(r+1)*64).  Channel-major DRAM layout
    keeps the outer AP dim at 128 so the DMA splits across all 16 SDMA slots."""
    return dram.ap() if c is None else dram.ap()[c]


def _emit(nc, tc, t):
    with ExitStack() as ctx:
        # Pools open for the whole kernel (small persistents only).
        cpool = ctx.enter_context(tc.tile_pool(name="consts", bufs=1))
        perm = ctx.enter_context(tc.tile_pool(name="perm", bufs=1))
        psp = ctx.enter_context(tc.tile_pool(name="psum", bufs=2, space="PSUM"))

        wb = cpool.tile([P, NCL], f32)
        nc.sync.dma_start(wb[:], t["wb"].ap())
        m128 = cpool.tile([P, P], f32)
        nc.sync.dma_start(m128[:], t["m128"].ap())
        c005 = cpool.tile([P, 1], f32)
        nc.vector.memset(c005[:], 0.05)

        score = perm.tile([P, F], f32)      # prev_scores -> decayed -> score_state
        wm = perm.tile([P, F], u8)
        tbits = perm.tile([P, 1], i32)

        # ---- Pass A (scoped pools: space returns before Pass B) ----
        with tc.tile_pool(name="cp", bufs=4) as cp_pool, \
             tc.tile_pool(name="pa", bufs=1) as apool:
            bg_t = apool.tile([P, F], f32)
            nc.sync.dma_start(bg_t[:], _pair_ap(t["bg"]))
            nc.scalar.dma_start(score[:], _pair_ap(t["ps"]))

            imp = apool.tile([P, F], f32)
            m8 = apool.tile([P, F], f32)
            for c in range(NCL):
                cpt = cp_pool.tile([P, F], f32, tag="cp")
                dma_eng = nc.sync if c % 2 == 0 else nc.scalar
                dma_eng.dma_start(cpt[:], _pair_ap(t["cp"], c))
                if c == 0:
                    nc.vector.tensor_tensor(
                        imp[:], cpt[:], wb[:, 0:1].broadcast_to([P, F]), Alu.mult
                    )
                    nc.scalar.copy(m8[:], cpt[:])
                else:
                    nc.vector.scalar_tensor_tensor(
                        out=imp[:], in0=cpt[:], scalar=wb[:, c : c + 1], in1=imp[:],
                        op0=Alu.mult, op1=Alu.add,
                    )
                    nc.vector.tensor_tensor(m8[:], m8[:], cpt[:], Alu.max)

            gate = apool.tile([P, F], f32)
            nc.vector.tensor_tensor(gate[:], m8[:], bg_t[:], Alu.is_gt)
            nc.vector.tensor_tensor(imp[:], imp[:], gate[:], Alu.mult)
            nc.sync.dma_start(_pair_ap(t["o_imp"]), imp[:])

            nc.scalar.mul(score[:], score[:], 0.9)      # score := decayed
            thr = apool.tile([P, F], f32)
            nc.scalar.activation(
                thr[:], score[:], mybir.ActivationFunctionType.Identity,
                bias=c005[:, 0:1], scale=1.0,
            )
            nc.vector.tensor_tensor(wm[:], imp[:], thr[:], Alu.is_gt)
            nc.scalar.dma_start(_pair_ap(t["o_wm"]), wm[:])
            # score := score_state = where(wm, imp, decayed)
            nc.vector.copy_predicated(score[:], wm[:], imp[:])
            nc.sync.dma_start(_pair_ap(t["o_score"]), score[:])

        # ---- Pass B + top-k select + Pass C ----
        with tc.tile_pool(name="ms", bufs=1) as msp, \
             tc.tile_pool(name="cur", bufs=2) as curp, \
             tc.tile_pool(name="bs", bufs=1) as bsp, \
             tc.tile_pool(name="bst", bufs=2) as bstp:
            # Pass B: memory_state (independent of the top-k select)
            ms_tiles = []
            for c in range(C):
                ms_c = msp.tile([P, F], f32, tag=f"ms{c}")
                nc.sync.dma_start(ms_c[:], _pair_ap(t["pm"], c))
                cur_t = curp.tile([P, F], f32, tag="cur")
                nc.scalar.dma_start(cur_t[:], _pair_ap(t["cur"], c))
                nc.vector.copy_predicated(ms_c[:], wm[:], cur_t[:])
                nc.sync.dma_start(_pair_ap(t["o_ms"], c), ms_c[:])
                ms_tiles.append(ms_c)

            # Exact per-row top-k threshold: MSB-first radix descent.
            # Phase 1 resolves T bits 29..14 on uint16 codes (x_bits >> 14;
            # order-exact since cand low bits are 0) at 2 elem/cycle; phase 2
            # resolves bits 13..0 with f32 compares on the scores.  Each round:
            # fused compare+per-partition-count (scalar_tensor_tensor), one
            # constant group-sum matmul -> PSUM row totals on every partition,
            # then bitwise threshold update.  All counts are exact integers.
            shf = bsp.tile([P, F], i32, tag="shf")
            nc.vector.tensor_scalar(shf[:], score[:].bitcast(i32), 14, None,
                                    Alu.logical_shift_right)
            code16 = bsp.tile([P, F], mybir.dt.uint16, tag="code16")
            nc.vector.tensor_copy(code16[:], shf[:])
            nbits = int(os.environ.get("KERNEL_NBITS", "30"))
            if nbits < 30:
                nc.vector.memset(tbits[:], 0x3F340000)  # debug: skip descent tail
            else:
                nc.vector.memset(tbits[:], 0)

            def count_round(b, in_tile, scalar_ap, scr_tag, scr_dt):
                scratch = bsp.tile([P, F], scr_dt, tag=scr_tag)
                cpart = bstp.tile([P, 1], f32, tag="cpart")
                nc.vector.scalar_tensor_tensor(
                    out=scratch[:], in0=in_tile[:], scalar=scalar_ap,
                    in1=in_tile[:], op0=Alu.is_ge, op1=Alu.bypass,
                    accum_out=cpart[:],
                )
                pball = psp.tile([P, 1], f32, tag="pball")
                nc.tensor.matmul(pball[:], m128[:], cpart[:], start=True, stop=True)
                cond = bstp.tile([P, 1], i32, tag="cond")
                nc.vector.tensor_scalar(cond[:], pball[:], float(KEEP), None, Alu.is_ge)
                bitv = bstp.tile([P, 1], i32, tag="bitv")
                nc.vector.tensor_scalar(bitv[:], cond[:], b, None, Alu.logical_shift_left)
                nc.vector.tensor_tensor(tbits[:], tbits[:], bitv[:], Alu.bitwise_or)

            if nbits >= 30:
                for b in range(15, -1, -1):  # T bits 29..14 == code16 bits 15..0
                    cand = bstp.tile([P, 1], i32, tag="cand")
                    nc.vector.tensor_scalar(cand[:], tbits[:], 1 << b, None, Alu.bitwise_or)
                    count_round(b, code16, cand[:].bitcast(mybir.dt.uint16)[:, 0:1],
                                "scr16", mybir.dt.uint16)
                nc.vector.tensor_scalar(tbits[:], tbits[:], 14, None,
                                        Alu.logical_shift_left)
                for b in range(13, -1, -1):  # T bits 13..0 on f32 scores
                    cand = bstp.tile([P, 1], i32, tag="cand")
                    nc.vector.tensor_scalar(cand[:], tbits[:], 1 << b, None, Alu.bitwise_or)
                    count_round(b, score, cand[:].bitcast(f32), "scr", f32)

            # Pass C: output mask + memory_image
            maskf = bsp.tile([P, F], f32, tag="maskf")
            nc.vector.tensor_tensor(
                maskf[:], score[:], tbits[:].bitcast(f32).broadcast_to([P, F]), Alu.is_ge
            )
            om8 = bsp.tile([P, F], u8, tag="om8")
            nc.scalar.copy(om8[:], maskf[:])
            nc.scalar.dma_start(_pair_ap(t["o_om"]), om8[:])
            for c in range(C):
                ms_c = ms_tiles[c]
                nc.vector.tensor_tensor(ms_c[:], ms_c[:], maskf[:], Alu.mult)
                dma_eng = nc.sync if c % 2 == 0 else nc.scalar
                dma_eng.dma_start(_pair_ap(t["o_mi"], c), ms_c[:])


def build():
    if "nc" in _cached:
        return _cached["nc"]
    nc = bacc.Bacc("TRN2", target_bir_lowering=False, debug=False, enable_asserts=True)
    t = {
        "cur": nc.dram_tensor("cur", [C, P, F], f32, kind="ExternalInput"),
        "cp": nc.dram_tensor("cp", [NCL, P, F], f32, kind="ExternalInput"),
        "bg": nc.dram_tensor("bg", [P, F], f32, kind="ExternalInput"),
        "pm": nc.dram_tensor("pm", [C, P, F], f32, kind="ExternalInput"),
        "ps": nc.dram_tensor("ps", [P, F], f32, kind="ExternalInput"),
        "wb": nc.dram_tensor("wb", [P, NCL], f32, kind="ExternalInput"),
        "m128": nc.dram_tensor("m128", [P, P], f32, kind="ExternalInput"),
        "o_mi": nc.dram_tensor("o_mi", [C, P, F], f32, kind="ExternalOutput"),
        "o_ms": nc.dram_tensor("o_ms", [C, P, F], f32, kind="ExternalOutput"),
        "o_score": nc.dram_tensor("o_score", [P, F], f32, kind="ExternalOutput"),
        "o_imp": nc.dram_tensor("o_imp", [P, F], f32, kind="ExternalOutput"),
        "o_wm": nc.dram_tensor("o_wm", [P, F], u8, kind="ExternalOutput"),
        "o_om": nc.dram_tensor("o_om", [P, F], u8, kind="ExternalOutput"),
    }
    with tile.TileContext(nc) as tc:
        _emit(nc, tc, t)
    nc.compile()
    _cached["nc"] = nc
    return nc


def make_in_maps(current_image, class_probs, background_prob, prev_memory,
                 prev_scores, class_weights):
    wb = np.ascontiguousarray(
        np.broadcast_to(class_weights.reshape(1, NCL).astype(np.float32), (P, NCL))
    )
    # group-sum matrix: m128[q, p] = 1 iff q and p belong to the same batch row
    m128 = np.zeros((P, P), np.float32)
    for r in range(RPC):
        m128[r * PR : (r + 1) * PR, r * PR : (r + 1) * PR] = 1.0
    in_maps = []
    for i in range(NCORES):
        b0 = i * RPC
        sl = slice(b0, b0 + RPC)
        def chmajor(x, nch):
            # [RPC, nch, 512, 512] -> [nch, RPC*PR, F] channel-major contiguous
            return np.ascontiguousarray(
                x.reshape(RPC, nch, P // RPC, F).transpose(1, 0, 2, 3)
            ).reshape(nch, P, F)

        in_maps.append({
            "cur": chmajor(current_image[sl], C),
            "cp": chmajor(class_probs[sl], NCL),
            "bg": np.ascontiguousarray(background_prob[sl]).reshape(P, F),
            "pm": chmajor(prev_memory[sl], C),
            "ps": np.ascontiguousarray(prev_scores[sl]).reshape(P, F),
            "wb": wb, "m128": m128,
        })
    return in_maps


def assemble(results):
    def gather(name, chans, dtype=None):
        parts = []
        for i in range(NCORES):
            a = results[i][name]
            if chans > 1:
                # [chans, P, F] channel-major -> [RPC, chans, 512, 512]
                a = a.reshape(chans, RPC, PR, F).transpose(1, 0, 2, 3)
            a = a.reshape(RPC, chans, H, W)
            parts.append(a)
        out = np.concatenate(parts, axis=0)
        return out if dtype is None else out.astype(dtype)

    return (
        gather("o_mi", C),
        gather("o_ms", C),
        gather("o_score", 1),
        gather("o_imp", 1),
        gather("o_wm", 1, np.bool_),
        gather("o_om", 1, np.bool_),
    )


def kernel(current_image, class_probs, background_prob, prev_memory,
           prev_scores, class_weights, _trace=False):
    nc = build()
    in_maps = make_in_maps(current_image, class_probs, background_prob,
                           prev_memory, prev_scores, class_weights)
    res = bass_utils.run_bass_kernel_spmd(
        nc, in_maps, core_ids=list(range(NCORES)),
        trace=_trace or bool(os.environ.get("KERNEL_TRACE")),
    )
    _cached["last_result"] = res
    return assemble(res.results)
